# revision 1
# baseline (speedup 1.0000x reference)
"""Trainium2 Bass kernel for AttentionFusionModel (B=4, S=4096, D=200).

out = (attn(x1) + attn(x2)) @ Wo.T + bo, with attn sharing Wq/Wk/Wv.

Sharding: 8 (batch, modality) pairs -> 8 NeuronCores, one full self-attention
per core. Modality fusion = pairwise ReduceScatter between cores (2b, 2b+1),
each core projecting its own attention output first (projection is linear, so
proj(a1 + a2) = proj(a1) + proj(a2); softmax row-normalization commutes with
the projection and is applied post-projection as a per-row scale). The RS is
chunked so it overlaps the tail of the attention compute.

Per-core layout strategy (all big matmuls in bf16, fp32 PSUM accumulate):
  X^T [201, S]   (d on partitions, +ones row)  via DMA-transpose (bf16 xbar)
  Q^T, K^T [200, S] = (W·sc)^T-stationary matmuls (bias via ones-row aug)
  V [S, 201]     natural layout, +ones column (for sumexp)
  scores^T[k,q] tiles = K^T-slice.T @ Q^T   (contract d: 128+72 blocks)
  expT = Exp(scores^T)  on ScalarE (no max subtraction; |scores| ~< 7)
  U^T[d+1, q] += V'[k,:].T @ expT           (row 200 = sumexp L)
  proj[q, 201] = U^T-slice.T @ Wo'^T        (col 200 = L passthrough)
  out rows = proj[:, :200] * (1/L) + bo/2
"""

import os
import sys

sys.path.insert(0, "/opt/trn_rl_repo")

import numpy as np
from contextlib import ExitStack

from concourse import bacc, mybir, tile
from concourse.bass_utils import run_bass_kernel_spmd
from concourse.masks import make_identity

F32 = mybir.dt.float32
BF16 = mybir.dt.bfloat16
AF = mybir.ActivationFunctionType
ALU = mybir.AluOpType

B = 4
S = 4096
D = 200
P = 128
D2 = D - P            # 72
DA = D + 1            # 201 (augmented with ones row / sumexp col)
NCORES = 8
RG = [[0, 1], [2, 3], [4, 5], [6, 7]]  # core 2b+m handles (batch b, modality m)

USE_RS = True


def _emit_av(nc, acc1, acc2, v_sb, et, kb, nkb):
    k0 = kb * DA
    st = kb == 0
    sp = kb == nkb - 1
    nc.tensor.matmul(acc1[:], v_sb[:, k0:k0 + P], et[:], start=st, stop=sp)
    nc.tensor.matmul(acc2[:], v_sb[:, k0 + P:k0 + DA], et[:], start=st, stop=sp)


def _emit(ctx, tc, nc, exts, s_len, use_rs):
    x_ext, wq_ext, wk_ext, wv_ext, wo_ext, bo_ext, out_ext, rs_bufs = exts
    QG = min(512, s_len)
    nkb = s_len // P
    nqg = s_len // QG
    qg_per_chunk = min(2, nqg)
    nchunk = nqg // qg_per_chunk
    crows = qg_per_chunk * QG  # rs chunk input rows

    pers = ctx.enter_context(tc.tile_pool(name="pers", bufs=1))
    xt0 = pers.tile([P, s_len], BF16)
    xt1 = pers.tile([P, s_len], BF16)       # rows 0:72 = d 128:200, 72 = ones
    qt0 = pers.tile([P, s_len], BF16)
    qt1 = pers.tile([D2, s_len], BF16)
    kt0 = pers.tile([P, s_len], BF16)
    kt1 = pers.tile([D2, s_len], BF16)
    v_sb = pers.tile([P, nkb * DA], BF16)   # nkb tiles of [128, 201], col 200 = 1s
    wq0 = pers.tile([P, D], BF16)
    wq1 = pers.tile([D2 + 1, D], BF16)
    wk0 = pers.tile([P, D], BF16)
    wk1 = pers.tile([D2 + 1, D], BF16)
    wv0 = pers.tile([P, DA], BF16)
    wv1 = pers.tile([D2 + 1, DA], BF16)
    wo0 = pers.tile([P, DA], BF16)
    wo1 = pers.tile([D2 + 1, DA], BF16)
    bo_sb = pers.tile([P, D], F32)

    # ---- phase 1: load X, build X^T via PE transposes (bf16, 1 cyc/row) ----
    ident = pers.tile([P, P], BF16)
    make_identity(nc, ident[:])
    # ones row lives at partition 72 of xt1; engine APs need 32-aligned
    # partition bases, so memset [64:128) and let the X^T copies overwrite
    # rows 64..71 (rows 73.. stay harmless junk, never read).
    nc.vector.memset(xt1[64:P, :], 1.0)
    with ExitStack() as ph1:
        xp = ph1.enter_context(tc.tile_pool(name="xp", bufs=8))
        tps = ph1.enter_context(tc.tile_pool(name="tps", bufs=2, space="PSUM"))
        for n in range(nkb):
            c0, c1 = n * P, (n + 1) * P
            x_in = xp.tile([P, D], F32, tag="xin")
            nc.sync.dma_start(out=x_in[:], in_=x_ext[c0:c1, :])
            xc = xp.tile([P, D], BF16, tag="xc")
            nc.vector.tensor_copy(xc[:], x_in[:])
            p1 = tps.tile([P, P], BF16, tag="tp1")
            nc.tensor.transpose(p1[:], xc[:, 0:P], ident[:])
            p2 = tps.tile([D2, P], BF16, tag="tp2")
            nc.tensor.transpose(p2[:], xc[:, P:D], ident[:])
            nc.vector.tensor_copy(xt0[:, c0:c1], p1[:])
            nc.vector.tensor_copy(xt1[0:D2, c0:c1], p2[:])

        nc.sync.dma_start(out=bo_sb[:], in_=bo_ext[:, :])
        wst = ph1.enter_context(tc.tile_pool(name="wstage", bufs=2))
        for (ext, b0, b1, width) in [
            (wq_ext, wq0, wq1, D),
            (wk_ext, wk0, wk1, D),
            (wv_ext, wv0, wv1, DA),
            (wo_ext, wo0, wo1, DA),
        ]:
            wf0 = wst.tile([P, DA], F32, tag="wf0")
            wf1 = wst.tile([D2 + 1, DA], F32, tag="wf1")
            nc.sync.dma_start(out=wf0[:, 0:width], in_=ext[0:P, :])
            nc.sync.dma_start(out=wf1[:, 0:width], in_=ext[P:DA, :])
            nc.vector.tensor_copy(b0[:, 0:width], wf0[:, 0:width])
            nc.vector.tensor_copy(b1[:, 0:width], wf1[:, 0:width])

        # ---- QKV projections ----
        qkps = ph1.enter_context(tc.tile_pool(name="qkps", bufs=2, space="PSUM"))
        CH = min(512, s_len)
        for (w0, w1, t0, t1) in [(wq0, wq1, qt0, qt1), (wk0, wk1, kt0, kt1)]:
            for ob, obw in [(0, P), (1, D2)]:
                tdst = t0 if ob == 0 else t1
                for ch in range(s_len // CH):
                    c0, c1 = ch * CH, (ch + 1) * CH
                    ps = qkps.tile([P, CH], F32, tag="qk")
                    nc.tensor.matmul(ps[0:obw, :], w0[:, ob * P:ob * P + obw],
                                     xt0[:, c0:c1], start=True, stop=False)
                    nc.tensor.matmul(ps[0:obw, :], w1[:, ob * P:ob * P + obw],
                                     xt1[0:D2 + 1, c0:c1], start=False, stop=True)
                    nc.vector.tensor_copy(tdst[:, c0:c1], ps[0:obw, :])

        vps = ph1.enter_context(tc.tile_pool(name="vps", bufs=2, space="PSUM"))
        for n in range(nkb):
            c0, c1 = n * P, (n + 1) * P
            pv = vps.tile([P, DA], F32, tag="pv")
            nc.tensor.matmul(pv[:], xt0[:, c0:c1], wv0[:], start=True, stop=False)
            nc.tensor.matmul(pv[:], xt1[0:D2 + 1, c0:c1], wv1[:],
                             start=False, stop=True)
            nc.vector.tensor_copy(v_sb[:, n * DA:(n + 1) * DA], pv[:])

    # ---- phase 2: attention + projection + epilogue (+ chunked RS) ----
    with ExitStack() as ph2:
        scp = ph2.enter_context(tc.tile_pool(name="scp", bufs=2, space="PSUM"))
        accp = ph2.enter_context(tc.tile_pool(name="accp", bufs=2, space="PSUM"))
        projp = ph2.enter_context(tc.tile_pool(name="projp", bufs=2, space="PSUM"))
        etp = ph2.enter_context(tc.tile_pool(name="etp", bufs=3))
        utp = ph2.enter_context(tc.tile_pool(name="utp", bufs=2))
        epip = ph2.enter_context(tc.tile_pool(name="epip", bufs=4))
        for qg in range(nqg):
            q0, q1 = qg * QG, (qg + 1) * QG
            acc1 = accp.tile([P, QG], F32, tag="acc1")
            acc2 = accp.tile([D2 + 1, QG], F32, tag="acc2")
            ets = {}
            for kb in range(nkb):
                k0 = kb * P
                sc_ps = scp.tile([P, QG], F32, tag="sc")
                nc.tensor.matmul(sc_ps[:], kt0[:, k0:k0 + P], qt0[:, q0:q1],
                                 start=True, stop=False)
                nc.tensor.matmul(sc_ps[:], kt1[:, k0:k0 + P], qt1[:, q0:q1],
                                 start=False, stop=True)
                et = etp.tile([P, QG], BF16, tag="et")
                nc.scalar.activation(et[:], sc_ps[:], AF.Exp)
                ets[kb] = et
                if kb >= 1:
                    _emit_av(nc, acc1, acc2, v_sb, ets.pop(kb - 1), kb - 1, nkb)
            _emit_av(nc, acc1, acc2, v_sb, ets.pop(nkb - 1), nkb - 1, nkb)

            ut0 = utp.tile([P, QG], BF16, tag="ut0")
            ut1 = utp.tile([D2 + 1, QG], BF16, tag="ut1")
            nc.vector.tensor_copy(ut0[:], acc1[:])
            nc.vector.tensor_copy(ut1[:], acc2[:])
            chunk = qg // qg_per_chunk
            for qb in range(QG // P):
                pp = projp.tile([P, DA], F32, tag="pp")
                nc.tensor.matmul(pp[:], ut0[:, qb * P:(qb + 1) * P], wo0[:],
                                 start=True, stop=False)
                nc.tensor.matmul(pp[:], ut1[:, qb * P:(qb + 1) * P], wo1[:],
                                 start=False, stop=True)
                rc = epip.tile([P, 1], F32, tag="rc")
                nc.vector.reciprocal(rc[:], pp[:, D:DA])
                ot = epip.tile([P, D], F32, tag="ot")
                nc.vector.tensor_scalar(ot[:], pp[:, 0:D], rc[:], None, ALU.mult)
                nc.vector.tensor_tensor(ot[:], ot[:], bo_sb[:], ALU.add)
                r0 = q0 + qb * P
                if use_rs:
                    dst = rs_bufs[chunk][0]
                    nc.sync.dma_start(
                        out=dst[r0 - chunk * crows:r0 - chunk * crows + P, :],
                        in_=ot[:])
                else:
                    nc.sync.dma_start(out=out_ext[r0:r0 + P, :], in_=ot[:])

            if use_rs and (qg + 1) % qg_per_chunk == 0:
                ci, co = rs_bufs[chunk]
                nc.gpsimd.collective_compute(
                    "ReduceScatter",
                    ALU.add,
                    replica_groups=RG,
                    ins=[ci[:, :].opt()],
                    outs=[co[:, :].opt()],
                )
                orow = chunk * (crows // 2)
                nc.sync.dma_start(out=out_ext[orow:orow + crows // 2, :],
                                  in_=co[:, :])


_CACHE = {}


def _build(s_len=S, use_rs=USE_RS):
    key = (s_len, use_rs)
    if key not in _CACHE:
        nc = bacc.Bacc("TRN2", target_bir_lowering=False, debug=False,
                       num_devices=NCORES)
        x_ext = nc.dram_tensor("x", [s_len, D], F32, kind="ExternalInput")
        wq_ext = nc.dram_tensor("wq", [DA, D], F32, kind="ExternalInput")
        wk_ext = nc.dram_tensor("wk", [DA, D], F32, kind="ExternalInput")
        wv_ext = nc.dram_tensor("wv", [DA, DA], F32, kind="ExternalInput")
        wo_ext = nc.dram_tensor("wo", [DA, DA], F32, kind="ExternalInput")
        bo_ext = nc.dram_tensor("bo", [P, D], F32, kind="ExternalInput")
        out_rows = s_len // 2 if use_rs else s_len
        out_ext = nc.dram_tensor("out", [out_rows, D], F32, kind="ExternalOutput")
        rs_bufs = []
        if use_rs:
            QG = min(512, s_len)
            nqg = s_len // QG
            qg_per_chunk = min(2, nqg)
            nchunk = nqg // qg_per_chunk
            crows = qg_per_chunk * QG
            for g in range(nchunk):
                ci = nc.dram_tensor(f"rs_in{g}", [crows, D], F32)
                co = nc.dram_tensor(f"rs_out{g}", [crows // 2, D], F32)
                rs_bufs.append((ci, co))
        exts = (x_ext, wq_ext, wk_ext, wv_ext, wo_ext, bo_ext, out_ext, rs_bufs)
        with tile.TileContext(nc) as tc:
            with ExitStack() as ctx:
                _emit(ctx, tc, nc, exts, s_len, use_rs)
        nc.compile()
        _CACHE[key] = nc
    return _CACHE[key]


def _prep_in_maps(m1, m2, Wq, bq, Wk, bk, Wv, bv, Wo, bo, s_len=S):
    sc = np.float32(1.0 / np.sqrt(D))
    wq_p = np.zeros((DA, D), np.float32)
    wq_p[:D] = Wq.T * sc
    wq_p[D] = bq * sc
    wk_p = np.zeros((DA, D), np.float32)
    wk_p[:D] = Wk.T
    wk_p[D] = bk
    wv_p = np.zeros((DA, DA), np.float32)
    wv_p[:D, :D] = Wv.T
    wv_p[D, :D] = bv
    wv_p[D, D] = 1.0
    wo_p = np.zeros((DA, DA), np.float32)
    wo_p[:D, :D] = Wo.T
    wo_p[D, D] = 1.0
    bo_t = np.ascontiguousarray(
        np.broadcast_to((bo * 0.5).astype(np.float32), (P, D)))
    in_maps = []
    for c in range(NCORES):
        b, m = c // 2, c % 2
        x = (m1 if m == 0 else m2)[b][:s_len]
        in_maps.append({
            "x": np.ascontiguousarray(x, np.float32),
            "wq": wq_p, "wk": wk_p, "wv": wv_p, "wo": wo_p, "bo": bo_t,
        })
    return in_maps


def _run(inputs, s_len=S, use_rs=USE_RS, trace=False, tmpdir=None):
    m1 = np.asarray(inputs["modal1_input"], np.float32)
    m2 = np.asarray(inputs["modal2_input"], np.float32)
    args = [np.asarray(inputs[k], np.float32)
            for k in ("Wq", "bq", "Wk", "bk", "Wv", "bv", "Wo", "bo")]
    nc = _build(s_len, use_rs)
    in_maps = _prep_in_maps(m1, m2, *args, s_len=s_len)
    kr = run_bass_kernel_spmd(nc, in_maps, core_ids=list(range(NCORES)),
                              trace=trace, tmpdir=tmpdir)
    res = kr.results
    out = np.empty((B, s_len, D), np.float32)
    if use_rs:
        # chunked RS: core 2b holds the first half of every chunk, core 2b+1
        # the second half; chunk g covers global rows [g*crows, (g+1)*crows)
        QG = min(512, s_len)
        nqg = s_len // QG
        crows = min(2, nqg) * QG
        csz = crows // 2
        nchunk = s_len // crows
        for b in range(B):
            for g in range(nchunk):
                lo, hi = g * csz, (g + 1) * csz
                out[b, g * crows:g * crows + csz] = res[2 * b]["out"][lo:hi]
                out[b, g * crows + csz:(g + 1) * crows] = \
                    res[2 * b + 1]["out"][lo:hi]
    else:
        for b in range(B):
            out[b] = res[2 * b]["out"] + res[2 * b + 1]["out"]
    return out, kr


def kernel(**inputs):
    out, _ = _run(inputs)
    return out



# revision 2
# speedup vs baseline: 1.0282x; 1.0282x over previous
"""Trainium2 Bass kernel for AttentionFusionModel (B=4, S=4096, D=200).

out = (attn(x1) + attn(x2)) @ Wo.T + bo, with attn sharing Wq/Wk/Wv.

Sharding: 8 (batch, modality) pairs -> 8 NeuronCores, one full self-attention
per core. Modality fusion = pairwise ReduceScatter between cores (2b, 2b+1),
each core projecting its own attention output first (projection is linear, so
proj(a1 + a2) = proj(a1) + proj(a2); softmax row-normalization commutes with
the projection and is applied post-projection as a per-row scale). The RS is
chunked per q-group (512 rows, bf16) so only the last small chunk is exposed.

Per-core layout strategy (all big matmuls in bf16, fp32 PSUM accumulate):
  X^T [201, S]   fed pre-transposed+bf16 from host (d on partitions, +ones row)
  Q^T, K^T [200, S] = (W·sc)^T-stationary matmuls (bias via ones-row aug)
  V [S, 201]     natural layout, +ones column (for sumexp)
  scores^T[k,q] tiles = K^T-slice.T @ Q^T   (contract d: 128+72 blocks)
  expT = Exp(scores^T)  on ScalarE (no max subtraction; |scores| ~< 7)
  U^T[d+1, q] += V'[k,:].T @ expT           (row 200 = sumexp L)
  proj[q, 201] = U^T-slice.T @ Wo'^T        (col 200 = L passthrough;
                                             L row of Wo' carries bo/2 so the
                                             bias rides the projection)
  out rows = proj[:, :200] * (1/L)  (bf16)

A burst of dummy matmuls at kernel start keeps the PE HAM clock-gate warm
(2.4 GHz) through the QKV phase instead of ramping ~50us into the kernel.
"""

import sys

sys.path.insert(0, "/opt/trn_rl_repo")

import numpy as np
from contextlib import ExitStack

import ml_dtypes

from concourse import bacc, mybir, tile
from concourse.bass_utils import run_bass_kernel_spmd

F32 = mybir.dt.float32
BF16 = mybir.dt.bfloat16
AF = mybir.ActivationFunctionType
ALU = mybir.AluOpType
NP_BF16 = np.dtype(ml_dtypes.bfloat16)

B = 4
S = 4096
D = 200
P = 128
D2 = D - P            # 72
DA = D + 1            # 201 (augmented with ones row / sumexp col)
WCOL = 2 * D + 2 * DA  # packed weight columns: wq|wk|wv|wo
NCORES = 8
RG = [[0, 1], [2, 3], [4, 5], [6, 7]]  # core 2b+m handles (batch b, modality m)

USE_RS = True
NWARM = 150


def _emit_av(nc, acc1, acc2, v_sb, et, kb, nkb):
    k0 = kb * DA
    st = kb == 0
    sp = kb == nkb - 1
    nc.tensor.matmul(acc1[:], v_sb[:, k0:k0 + P], et[:], start=st, stop=sp)
    nc.tensor.matmul(acc2[:], v_sb[:, k0 + P:k0 + DA], et[:], start=st, stop=sp)


def _emit(ctx, tc, nc, exts, s_len, use_rs):
    xt_ext, whi_ext, wlo_ext, out_ext, rs_bufs = exts
    QG = min(512, s_len)
    nkb = s_len // P
    nqg = s_len // QG
    csz = QG // 2  # rs output rows per chunk

    pers = ctx.enter_context(tc.tile_pool(name="pers", bufs=1))
    xt0 = pers.tile([P, s_len], BF16)
    xt1 = pers.tile([D2 + 1, s_len], BF16)  # rows 0:72 = d 128:200, 72 = ones
    qt0 = pers.tile([P, s_len], BF16)
    qt1 = pers.tile([D2, s_len], BF16)
    kt0 = pers.tile([P, s_len], BF16)
    kt1 = pers.tile([D2, s_len], BF16)
    v_sb = pers.tile([P, nkb * DA], BF16)   # nkb tiles of [128, 201], col 200 = 1s
    whi = pers.tile([P, WCOL], BF16)
    wlo = pers.tile([D2 + 1, WCOL], BF16)
    wu = pers.tile([P, 64], BF16)

    wq0, wq1 = whi[:, 0:D], wlo[:, 0:D]
    wk0, wk1 = whi[:, D:2 * D], wlo[:, D:2 * D]
    wv0, wv1 = whi[:, 2 * D:2 * D + DA], wlo[:, 2 * D:2 * D + DA]
    wo0, wo1 = whi[:, 2 * D + DA:WCOL], wlo[:, 2 * D + DA:WCOL]

    # ---- phase 1: warmup + load + QKV projections ----
    with ExitStack() as ph1:
        wups = ph1.enter_context(tc.tile_pool(name="wups", bufs=1, space="PSUM"))
        nc.vector.memset(wu[:], 0.0)
        trash = wups.tile([P, 64], F32)
        for _ in range(NWARM):
            nc.tensor.matmul(trash[0:64, :], wu[:, 0:64], wu[:],
                             start=True, stop=True)

        nc.sync.dma_start(out=whi[:], in_=whi_ext[:, :])
        nc.sync.dma_start(out=wlo[:], in_=wlo_ext[:, :])
        DCH = min(1024, s_len)
        for ch in range(s_len // DCH):
            c0, c1 = ch * DCH, (ch + 1) * DCH
            nc.sync.dma_start(out=xt0[:, c0:c1], in_=xt_ext[0:P, c0:c1])
            nc.sync.dma_start(out=xt1[:, c0:c1], in_=xt_ext[P:DA, c0:c1])

        qkps = ph1.enter_context(tc.tile_pool(name="qkps", bufs=3, space="PSUM"))
        vps = ph1.enter_context(tc.tile_pool(name="vps", bufs=2, space="PSUM"))
        CH = min(512, s_len)
        for ch in range(s_len // CH):
            c0, c1 = ch * CH, (ch + 1) * CH
            for (w0, w1, t0, t1) in ((wq0, wq1, qt0, qt1), (wk0, wk1, kt0, kt1)):
                for ob, obw in ((0, P), (1, D2)):
                    tdst = t0 if ob == 0 else t1
                    ps = qkps.tile([P, CH], F32, tag="qk")
                    nc.tensor.matmul(ps[0:obw, :], w0[:, ob * P:ob * P + obw],
                                     xt0[:, c0:c1], start=True, stop=False)
                    nc.tensor.matmul(ps[0:obw, :], w1[:, ob * P:ob * P + obw],
                                     xt1[:, c0:c1], start=False, stop=True)
                    nc.vector.tensor_copy(tdst[:, c0:c1], ps[0:obw, :])
            for n in range(ch * (CH // P), (ch + 1) * (CH // P)):
                pv = vps.tile([P, DA], F32, tag="pv")
                nc.tensor.matmul(pv[:], xt0[:, n * P:(n + 1) * P], wv0,
                                 start=True, stop=False)
                nc.tensor.matmul(pv[:], xt1[:, n * P:(n + 1) * P], wv1,
                                 start=False, stop=True)
                nc.vector.tensor_copy(v_sb[:, n * DA:(n + 1) * DA], pv[:])

    # ---- phase 2: attention + projection + epilogue (+ chunked RS) ----
    with ExitStack() as ph2:
        scp = ph2.enter_context(tc.tile_pool(name="scp", bufs=2, space="PSUM"))
        accp = ph2.enter_context(tc.tile_pool(name="accp", bufs=2, space="PSUM"))
        projp = ph2.enter_context(tc.tile_pool(name="projp", bufs=2, space="PSUM"))
        etp = ph2.enter_context(tc.tile_pool(name="etp", bufs=3))
        utp = ph2.enter_context(tc.tile_pool(name="utp", bufs=2))
        epip = ph2.enter_context(tc.tile_pool(name="epip", bufs=4))
        for qg in range(nqg):
            q0, q1 = qg * QG, (qg + 1) * QG
            acc1 = accp.tile([P, QG], F32, tag="acc1")
            acc2 = accp.tile([D2 + 1, QG], F32, tag="acc2")
            ets = {}
            for kb in range(nkb):
                k0 = kb * P
                sc_ps = scp.tile([P, QG], F32, tag="sc")
                nc.tensor.matmul(sc_ps[:], kt0[:, k0:k0 + P], qt0[:, q0:q1],
                                 start=True, stop=False)
                nc.tensor.matmul(sc_ps[:], kt1[:, k0:k0 + P], qt1[:, q0:q1],
                                 start=False, stop=True)
                et = etp.tile([P, QG], BF16, tag="et")
                nc.scalar.activation(et[:], sc_ps[:], AF.Exp)
                ets[kb] = et
                if kb >= 1:
                    _emit_av(nc, acc1, acc2, v_sb, ets.pop(kb - 1), kb - 1, nkb)
            _emit_av(nc, acc1, acc2, v_sb, ets.pop(nkb - 1), nkb - 1, nkb)

            ut0 = utp.tile([P, QG], BF16, tag="ut0")
            ut1 = utp.tile([D2 + 1, QG], BF16, tag="ut1")
            nc.vector.tensor_copy(ut0[:], acc1[:])
            nc.vector.tensor_copy(ut1[:], acc2[:])
            for qb in range(QG // P):
                pp = projp.tile([P, DA], F32, tag="pp")
                nc.tensor.matmul(pp[:], ut0[:, qb * P:(qb + 1) * P], wo0,
                                 start=True, stop=False)
                nc.tensor.matmul(pp[:], ut1[:, qb * P:(qb + 1) * P], wo1,
                                 start=False, stop=True)
                rc = epip.tile([P, 1], F32, tag="rc")
                nc.vector.reciprocal(rc[:], pp[:, D:DA])
                ot = epip.tile([P, D], BF16, tag="ot")
                nc.vector.tensor_scalar(ot[:], pp[:, 0:D], rc[:], None, ALU.mult)
                r0 = q0 + qb * P
                if use_rs:
                    dst = rs_bufs[qg][0]
                    nc.sync.dma_start(out=dst[qb * P:(qb + 1) * P, :], in_=ot[:])
                else:
                    nc.sync.dma_start(out=out_ext[r0:r0 + P, :], in_=ot[:])

            if use_rs:
                ci, co = rs_bufs[qg]
                nc.gpsimd.collective_compute(
                    "ReduceScatter",
                    ALU.add,
                    replica_groups=RG,
                    ins=[ci[:, :].opt()],
                    outs=[co[:, :].opt()],
                )
                nc.sync.dma_start(out=out_ext[qg * csz:(qg + 1) * csz, :],
                                  in_=co[:, :])


_CACHE = {}


def _build(s_len=S, use_rs=USE_RS):
    key = (s_len, use_rs)
    if key not in _CACHE:
        nc = bacc.Bacc("TRN2", target_bir_lowering=False, debug=False,
                       num_devices=NCORES)
        xt_ext = nc.dram_tensor("xt", [DA, s_len], BF16, kind="ExternalInput")
        whi_ext = nc.dram_tensor("whi", [P, WCOL], BF16, kind="ExternalInput")
        wlo_ext = nc.dram_tensor("wlo", [D2 + 1, WCOL], BF16,
                                 kind="ExternalInput")
        out_rows = s_len // 2 if use_rs else s_len
        out_ext = nc.dram_tensor("out", [out_rows, D], BF16,
                                 kind="ExternalOutput")
        rs_bufs = []
        if use_rs:
            QG = min(512, s_len)
            for g in range(s_len // QG):
                ci = nc.dram_tensor(f"rs_in{g}", [QG, D], BF16)
                co = nc.dram_tensor(f"rs_out{g}", [QG // 2, D], BF16)
                rs_bufs.append((ci, co))
        exts = (xt_ext, whi_ext, wlo_ext, out_ext, rs_bufs)
        with tile.TileContext(nc) as tc:
            with ExitStack() as ctx:
                _emit(ctx, tc, nc, exts, s_len, use_rs)
        nc.compile()
        _CACHE[key] = nc
    return _CACHE[key]


def _prep_in_maps(m1, m2, Wq, bq, Wk, bk, Wv, bv, Wo, bo, s_len=S):
    sc = np.float32(1.0 / np.sqrt(D))
    wpack = np.zeros((DA, WCOL), np.float32)
    wpack[:D, 0:D] = Wq.T * sc
    wpack[D, 0:D] = bq * sc
    wpack[:D, D:2 * D] = Wk.T
    wpack[D, D:2 * D] = bk
    wpack[:D, 2 * D:2 * D + D] = Wv.T
    wpack[D, 2 * D:2 * D + D] = bv
    wpack[D, 2 * D + D] = 1.0                  # ones col of V (sumexp)
    wo_c = 2 * D + DA
    wpack[:D, wo_c:wo_c + D] = Wo.T
    wpack[D, wo_c:wo_c + D] = bo * 0.5         # bias rides the L row (RS sums 2)
    wpack[D, wo_c + D] = 1.0                   # L passthrough
    whi = np.ascontiguousarray(wpack[:P]).astype(NP_BF16)
    wlo = np.ascontiguousarray(wpack[P:]).astype(NP_BF16)
    in_maps = []
    for c in range(NCORES):
        b, m = c // 2, c % 2
        x = np.asarray((m1 if m == 0 else m2)[b][:s_len], np.float32)
        xt = np.empty((DA, s_len), np.float32)
        xt[:D] = x.T
        xt[D] = 1.0
        in_maps.append({"xt": xt.astype(NP_BF16), "whi": whi, "wlo": wlo})
    return in_maps


def _run(inputs, s_len=S, use_rs=USE_RS, trace=False, tmpdir=None):
    m1 = np.asarray(inputs["modal1_input"], np.float32)
    m2 = np.asarray(inputs["modal2_input"], np.float32)
    args = [np.asarray(inputs[k], np.float32)
            for k in ("Wq", "bq", "Wk", "bk", "Wv", "bv", "Wo", "bo")]
    nc = _build(s_len, use_rs)
    in_maps = _prep_in_maps(m1, m2, *args, s_len=s_len)
    kr = run_bass_kernel_spmd(nc, in_maps, core_ids=list(range(NCORES)),
                              trace=trace, tmpdir=tmpdir)
    res = kr.results
    out = np.empty((B, s_len, D), np.float32)
    if use_rs:
        # chunked RS: core 2b holds the first half of every chunk, core 2b+1
        # the second half; chunk g covers global rows [g*QG, (g+1)*QG)
        QG = min(512, s_len)
        csz = QG // 2
        for b in range(B):
            for g in range(s_len // QG):
                lo, hi = g * csz, (g + 1) * csz
                out[b, g * QG:g * QG + csz] = \
                    np.asarray(res[2 * b]["out"][lo:hi], np.float32)
                out[b, g * QG + csz:(g + 1) * QG] = \
                    np.asarray(res[2 * b + 1]["out"][lo:hi], np.float32)
    else:
        for b in range(B):
            out[b] = (np.asarray(res[2 * b]["out"], np.float32)
                      + np.asarray(res[2 * b + 1]["out"], np.float32))
    return out, kr


def kernel(**inputs):
    out, _ = _run(inputs)
    return out


# revision 8
# speedup vs baseline: 1.0893x; 1.0594x over previous
"""Trainium2 Bass kernel for AttentionFusionModel (B=4, S=4096, D=200).

out = (attn(x1) + attn(x2)) @ Wo.T + bo, with attn sharing Wq/Wk/Wv.

Sharding: 8 (batch, modality) pairs -> 8 NeuronCores, one full self-attention
per core. Modality fusion = pairwise ReduceScatter between cores (2b, 2b+1).
Wo is folded into Wv on the host ((A@V)@Wo == A@(V@Wo), and the softmax
row-normalization commutes with the projection), so there is no separate
output-projection stage on device; bo/2 rides the sumexp (L) column.

Per-core layout strategy (all big matmuls in bf16, fp32 PSUM accumulate):
  X^T [201, S]   fed pre-transposed+bf16 from host (d on partitions, +ones row)
  Q^T, K^T [200, S] = (W·sc)^T-stationary matmuls (bias via ones-row aug)
  V2 [S, 201]    = X @ (Wv.T@Wo.T) + (bv@Wo.T + bo/2), natural layout,
                 col 200 = ones (sumexp L)
  scores^T[k,q] tiles = K^T-slice.T @ Q^T   (contract d: 128+72 blocks)
  expT = Exp(scores^T)  on ScalarE (no max subtraction; |scores| ~< 7)
  out[q, 201] += expT-slice.T @ V2[k,:]     (et stationary 128x128, V2 streams:
                                             full PE utilization; col 200 = L)
  out rows = out[:, :200] * (1/L)  (bf16)

A burst of dummy matmuls at kernel start keeps the PE HAM clock-gate warm
(2.4 GHz) through the QKV phase instead of ramping ~50us into the kernel.
"""

import sys

sys.path.insert(0, "/opt/trn_rl_repo")

import numpy as np
from contextlib import ExitStack

import ml_dtypes

from concourse import bacc, mybir, tile
from concourse.bass_utils import run_bass_kernel_spmd

F32 = mybir.dt.float32
BF16 = mybir.dt.bfloat16
AF = mybir.ActivationFunctionType
ALU = mybir.AluOpType
NP_BF16 = np.dtype(ml_dtypes.bfloat16)

B = 4
S = 4096
D = 200
P = 128
D2 = D - P            # 72
DA = D + 1            # 201 (augmented with ones row / sumexp col)
WCOL = 2 * D + DA     # packed weight columns: wq|wk|wv2
NCORES = 8
RG = [[0, 1], [2, 3], [4, 5], [6, 7]]  # core 2b+m handles (batch b, modality m)

USE_RS = True
NWARM = 100


def _emit(ctx, tc, nc, exts, s_len, use_rs):
    xt_ext, whi_ext, wlo_ext, out_ext, rs_bufs = exts
    QG = min(512, s_len)
    nkb = s_len // P
    nqg = s_len // QG
    nqb = QG // P
    csz = QG // 2  # rs output rows per chunk

    pers = ctx.enter_context(tc.tile_pool(name="pers", bufs=1))
    xt0 = pers.tile([P, s_len], BF16)
    xt1 = pers.tile([D2 + 1, s_len], BF16)  # rows 0:72 = d 128:200, 72 = ones
    qt0 = pers.tile([P, s_len], BF16)
    qt1 = pers.tile([D2, s_len], BF16)
    kt0 = pers.tile([P, s_len], BF16)
    kt1 = pers.tile([D2, s_len], BF16)
    v_sb = pers.tile([P, nkb * DA], BF16)   # nkb tiles of [128, 201], col 200 = 1s
    whi = pers.tile([P, WCOL], BF16)
    wlo = pers.tile([D2 + 1, WCOL], BF16)
    wu = pers.tile([P, 64], BF16)

    wq0, wq1 = whi[:, 0:D], wlo[:, 0:D]
    wk0, wk1 = whi[:, D:2 * D], wlo[:, D:2 * D]
    wv0, wv1 = whi[:, 2 * D:WCOL], wlo[:, 2 * D:WCOL]

    # ---- phase 1: warmup + load + QKV projections ----
    with ExitStack() as ph1:
        wups = ph1.enter_context(tc.tile_pool(name="wups", bufs=1, space="PSUM"))
        nc.vector.memset(wu[:], 0.0)
        trash = wups.tile([P, 64], F32)
        for _ in range(NWARM):
            nc.tensor.matmul(trash[0:64, :], wu[:, 0:64], wu[:],
                             start=True, stop=True)

        nc.scalar.dma_start(out=whi[:], in_=whi_ext[:, :])
        nc.scalar.dma_start(out=wlo[:], in_=wlo_ext[:, :])
        DCH = min(2048, s_len)
        for ch in range(s_len // DCH):
            c0, c1 = ch * DCH, (ch + 1) * DCH
            nc.sync.dma_start(out=xt0[:, c0:c1], in_=xt_ext[0:P, c0:c1])
            nc.gpsimd.dma_start(out=xt1[:, c0:c1], in_=xt_ext[P:DA, c0:c1])

        qkps = ph1.enter_context(tc.tile_pool(name="qkps", bufs=3, space="PSUM"))
        vps = ph1.enter_context(tc.tile_pool(name="vps", bufs=2, space="PSUM"))
        CH = min(512, s_len)
        for ch in range(s_len // CH):
            c0, c1 = ch * CH, (ch + 1) * CH
            for (w0, w1, t0, t1, cpeng) in ((wq0, wq1, qt0, qt1, 0),
                                            (wk0, wk1, kt0, kt1, 1)):
                for ob, obw in ((0, P), (1, D2)):
                    tdst = t0 if ob == 0 else t1
                    ps = qkps.tile([P, CH], F32, tag="qk")
                    nc.tensor.matmul(ps[0:obw, :], w0[:, ob * P:ob * P + obw],
                                     xt0[:, c0:c1], start=True, stop=False)
                    nc.tensor.matmul(ps[0:obw, :], w1[:, ob * P:ob * P + obw],
                                     xt1[:, c0:c1], start=False, stop=True)
                    if cpeng == 0:
                        nc.scalar.activation(tdst[:, c0:c1], ps[0:obw, :],
                                             AF.Copy)
                    else:
                        nc.vector.tensor_copy(tdst[:, c0:c1], ps[0:obw, :])
            for n in range(ch * (CH // P), (ch + 1) * (CH // P)):
                pv = vps.tile([P, DA], F32, tag="pv")
                nc.tensor.matmul(pv[:], xt0[:, n * P:(n + 1) * P], wv0,
                                 start=True, stop=False)
                nc.tensor.matmul(pv[:], xt1[:, n * P:(n + 1) * P], wv1,
                                 start=False, stop=True)
                nc.vector.tensor_copy(v_sb[:, n * DA:(n + 1) * DA], pv[:])

    # ---- phase 2: attention + epilogue (+ chunked RS) ----
    with ExitStack() as ph2:
        scp = ph2.enter_context(tc.tile_pool(name="scp", bufs=2, space="PSUM"))
        avp = ph2.enter_context(tc.tile_pool(name="avp", bufs=1, space="PSUM"))
        etp = ph2.enter_context(tc.tile_pool(name="etp", bufs=3))
        epip = ph2.enter_context(tc.tile_pool(name="epip", bufs=4))
        for qg in range(nqg):
            q0, q1 = qg * QG, (qg + 1) * QG
            # one full PSUM bank per qb: matmul start=True clears the whole
            # bank, so accumulation groups must not share one
            avs = [avp.tile([P, DA], F32, padded_shape=[P, 512],
                            tag=f"av{i}", name=f"av{i}") for i in range(nqb)]
            ets = {}

            def emit_av(kb):
                k0 = kb * DA
                st = kb == 0
                sp = kb == nkb - 1
                et = ets.pop(kb)
                for qb in range(nqb):
                    nc.tensor.matmul(avs[qb][:], et[:, qb * P:(qb + 1) * P],
                                     v_sb[:, k0:k0 + DA], start=st, stop=sp)

            for kb in range(nkb):
                k0 = kb * P
                sc_ps = scp.tile([P, QG], F32, tag="sc")
                nc.tensor.matmul(sc_ps[:], kt0[:, k0:k0 + P], qt0[:, q0:q1],
                                 start=True, stop=False)
                nc.tensor.matmul(sc_ps[:], kt1[:, k0:k0 + P], qt1[:, q0:q1],
                                 start=False, stop=True)
                et = etp.tile([P, QG], BF16, tag="et")
                nc.scalar.activation(et[:], sc_ps[:], AF.Exp)
                ets[kb] = et
                if kb >= 1:
                    emit_av(kb - 1)
            emit_av(nkb - 1)

            for qb in range(nqb):
                pp = avs[qb]
                rc = epip.tile([P, 1], F32, tag="rc")
                nc.vector.reciprocal(rc[:], pp[:, D:DA])
                ot = epip.tile([P, D], BF16, tag="ot")
                nc.vector.tensor_scalar(ot[:], pp[:, 0:D], rc[:], None, ALU.mult)
                r0 = q0 + qb * P
                if use_rs:
                    dst = rs_bufs[qg][0]
                    nc.sync.dma_start(out=dst[qb * P:(qb + 1) * P, :], in_=ot[:])
                else:
                    nc.sync.dma_start(out=out_ext[r0:r0 + P, :], in_=ot[:])

            if use_rs:
                ci, co = rs_bufs[qg]
                nc.gpsimd.collective_compute(
                    "ReduceScatter",
                    ALU.add,
                    replica_groups=RG,
                    ins=[ci[:, :].opt()],
                    outs=[co[:, :].opt()],
                )
                nc.sync.dma_start(out=out_ext[qg * csz:(qg + 1) * csz, :],
                                  in_=co[:, :])


_CACHE = {}


def _build(s_len=S, use_rs=USE_RS):
    key = (s_len, use_rs)
    if key not in _CACHE:
        nc = bacc.Bacc("TRN2", target_bir_lowering=False, debug=False,
                       num_devices=NCORES)
        xt_ext = nc.dram_tensor("xt", [DA, s_len], BF16, kind="ExternalInput")
        whi_ext = nc.dram_tensor("whi", [P, WCOL], BF16, kind="ExternalInput")
        wlo_ext = nc.dram_tensor("wlo", [D2 + 1, WCOL], BF16,
                                 kind="ExternalInput")
        out_rows = s_len // 2 if use_rs else s_len
        out_ext = nc.dram_tensor("out", [out_rows, D], BF16,
                                 kind="ExternalOutput")
        rs_bufs = []
        if use_rs:
            QG = min(512, s_len)
            for g in range(s_len // QG):
                ci = nc.dram_tensor(f"rs_in{g}", [QG, D], BF16)
                co = nc.dram_tensor(f"rs_out{g}", [QG // 2, D], BF16)
                rs_bufs.append((ci, co))
        exts = (xt_ext, whi_ext, wlo_ext, out_ext, rs_bufs)
        with tile.TileContext(nc) as tc:
            with ExitStack() as ctx:
                _emit(ctx, tc, nc, exts, s_len, use_rs)
        nc.compile()
        _CACHE[key] = nc
    return _CACHE[key]


def _prep_in_maps(m1, m2, Wq, bq, Wk, bk, Wv, bv, Wo, bo, s_len=S):
    sc = np.float32(1.0 / np.sqrt(D))
    wvo = (Wv.T @ Wo.T).astype(np.float32)          # x @ wvo == (x@Wv.T)@Wo.T
    bvo = (bv @ Wo.T + 0.5 * bo).astype(np.float32)  # RS sums two cores
    wpack = np.zeros((DA, WCOL), np.float32)
    wpack[:D, 0:D] = Wq.T * sc
    wpack[D, 0:D] = bq * sc
    wpack[:D, D:2 * D] = Wk.T
    wpack[D, D:2 * D] = bk
    wpack[:D, 2 * D:2 * D + D] = wvo
    wpack[D, 2 * D:2 * D + D] = bvo
    wpack[D, 2 * D + D] = 1.0                  # ones col of V2 (sumexp L)
    whi = np.ascontiguousarray(wpack[:P]).astype(NP_BF16)
    wlo = np.ascontiguousarray(wpack[P:]).astype(NP_BF16)
    in_maps = []
    for c in range(NCORES):
        b, m = c // 2, c % 2
        x = np.asarray((m1 if m == 0 else m2)[b][:s_len], np.float32)
        xt = np.empty((DA, s_len), np.float32)
        xt[:D] = x.T
        xt[D] = 1.0
        in_maps.append({"xt": xt.astype(NP_BF16), "whi": whi, "wlo": wlo})
    return in_maps


def _run(inputs, s_len=S, use_rs=USE_RS, trace=False, tmpdir=None):
    m1 = np.asarray(inputs["modal1_input"], np.float32)
    m2 = np.asarray(inputs["modal2_input"], np.float32)
    args = [np.asarray(inputs[k], np.float32)
            for k in ("Wq", "bq", "Wk", "bk", "Wv", "bv", "Wo", "bo")]
    nc = _build(s_len, use_rs)
    in_maps = _prep_in_maps(m1, m2, *args, s_len=s_len)
    kr = run_bass_kernel_spmd(nc, in_maps, core_ids=list(range(NCORES)),
                              trace=trace, tmpdir=tmpdir)
    res = kr.results
    out = np.empty((B, s_len, D), np.float32)
    if use_rs:
        # chunked RS: core 2b holds the first half of every chunk, core 2b+1
        # the second half; chunk g covers global rows [g*QG, (g+1)*QG)
        QG = min(512, s_len)
        csz = QG // 2
        for b in range(B):
            for g in range(s_len // QG):
                lo, hi = g * csz, (g + 1) * csz
                out[b, g * QG:g * QG + csz] = \
                    np.asarray(res[2 * b]["out"][lo:hi], np.float32)
                out[b, g * QG + csz:(g + 1) * QG] = \
                    np.asarray(res[2 * b + 1]["out"][lo:hi], np.float32)
    else:
        for b in range(B):
            out[b] = (np.asarray(res[2 * b]["out"], np.float32)
                      + np.asarray(res[2 * b + 1]["out"], np.float32))
    return out, kr


def kernel(**inputs):
    out, _ = _run(inputs)
    return out
